# revision 1
# baseline (speedup 1.0000x reference)
"""Local causal (sliding-window) attention kernel for Trainium2, SPMD over 8 NeuronCores.

Problem: x [1,4096,1024] -> QKV proj -> 16-head attention with causal window 64
         -> out proj. All fp32 at the interface.

Sharding: sequence-parallel. Core c owns queries [512c, 512c+512). Attention is
local (window 64), so each core only needs a 128-row key/value halo (the
previous 128-token block) in addition to its own 512 rows. Each core computes
its full output rows; host concatenates. No collectives.

All on-chip compute is bf16 (fp32 PSUM accumulation): host casts x/weights to
bf16, which halves HBM traffic (the projection phase is DMA-paced) and runs
every matmul at 1 cycle/row regardless of free-dim size. Measured end-to-end
rel err vs the fp32 reference is ~6e-3.

Attention is computed TRANSPOSED (S^T = K^T-stationary x Q): exp(S^T) is
already P^T, so no PE transposes and no P^T staging copies are needed. V tiles
carry an interleaved ones-block per head ([V_h | 1]), so each PV matmul also
accumulates the softmax denominators into psum rows 64:128 for free; the
normalization (x 1/denom) is fused into the psum->sbuf copy of the attention
output (DVE reciprocal + tensor_mul).

Per-core layouts (host pre-transposes so every DMA is a clean row-major tile):
  xT    [1024 d, 640 n]   x^T for rows [s-128, s+512) (core 0: first 128 zero)
  wqkvT [1024 d, 3072 o]  w_qkv^T
  woutT [1024 d, 1024 o]  w_out^T
  maskT [128, 768]        0/1 valid bits for the 5 S^T strips of one head
                          (per-core data; core 0 zeroes the kb0 strip)

S^T strips are trimmed to the query range that can see the strip's keys
(STRIP_Q, 768 columns total); psum banks A=[kb0, kb1, kb4], B=[kb2, kb3];
the sbuf P^T tile is [A | B] = [128, 768]. Strip kb holds keys
k = 128kb + row; entry (row, q) is valid iff q+65 <= k <= q+128.
"""

from contextlib import ExitStack

import ml_dtypes
import numpy as np

import concourse.bass as bass
import concourse.mybir as mybir
import concourse.tile as tile
from concourse import bacc
from concourse.bass_utils import run_bass_kernel_spmd

F32 = mybir.dt.float32
BF16 = mybir.dt.bfloat16
NP_BF16 = ml_dtypes.bfloat16


D_MODEL = 1024
N_HEADS = 16
D_HEAD = 64
WINDOW = 64
N_SEQ = 4096
N_CORES = 8
NLOC = N_SEQ // N_CORES          # 512 queries per core
HALO = 128                       # one full key block of halo
NTOT = NLOC + HALO               # 640 local rows (keys/values)
QB = 128                         # query block
NQB = NLOC // QB                 # 4 query blocks per core
KB = 128                         # key block
NKB = NTOT // KB                 # 5 key blocks per core
SCALE = 1.0 / np.sqrt(D_HEAD)

DT = D_MODEL // 128              # 8 contraction tiles

# Strips are trimmed to the queries that can see any key of the strip:
# kb0 -> q [0,64), kb1 -> [0,192), kb2 -> [128,320), kb3 -> [256,448),
# kb4 -> [384,512); 768 columns total (vs 1024 untrimmed).
STRIP_Q = {0: (0, 64), 1: (0, 192), 2: (128, 320), 3: (256, 448), 4: (384, 512)}
# P^T sbuf column offset of each strip kb (bank A cols 0:384, bank B 384:768)
STRIP_OFF = {0: 0, 1: 64, 4: 256, 2: 384, 3: 576}
STRIP_COLS = 768

# exposed for test.py (profiling info)
LAST_RESULT = None


def _emit_program(use_bqkv: bool, use_bout: bool, reps: int = 1) -> bass.Bass:
    # Bacc (not raw Bass): its finalize pipeline splits semaphore waits
    # (move_matmul_waits_to_ldweights / generate_event_semaphores) to satisfy
    # the HW limit of 1 sync wait per instruction.
    nc = bacc.Bacc()

    xT = nc.declare_dram_parameter("xT", [D_MODEL, NTOT], BF16, isOutput=False)
    wqkvT = nc.declare_dram_parameter("wqkvT", [D_MODEL, 3 * D_MODEL], BF16, isOutput=False)
    woutT = nc.declare_dram_parameter("woutT", [D_MODEL, D_MODEL], BF16, isOutput=False)
    maskT = nc.declare_dram_parameter("maskT", [128, STRIP_COLS], BF16, isOutput=False)
    if use_bqkv:
        bqkv = nc.declare_dram_parameter("bqkv", [1, 3 * D_MODEL], BF16, isOutput=False)
    if use_bout:
        bout = nc.declare_dram_parameter("bout", [1, D_MODEL], BF16, isOutput=False)
    y = nc.declare_dram_parameter("y", [NLOC, D_MODEL], F32, isOutput=True)

    with tile.TileContext(nc) as tc:
      for _rep in range(reps):
       with ExitStack() as ctx:
        consts = ctx.enter_context(tc.tile_pool(name="consts", bufs=1))
        xpool = ctx.enter_context(tc.tile_pool(name="xpool", bufs=1))
        wpool = ctx.enter_context(tc.tile_pool(name="wpool", bufs=18))
        qtp = ctx.enter_context(tc.tile_pool(name="qtp", bufs=1))
        ktp = ctx.enter_context(tc.tile_pool(name="ktp", bufs=1))
        vp = ctx.enter_context(tc.tile_pool(name="vp", bufs=1))
        aop = ctx.enter_context(tc.tile_pool(name="aop", bufs=1))
        work = ctx.enter_context(tc.tile_pool(name="work", bufs=2))
        outp = ctx.enter_context(tc.tile_pool(name="outp", bufs=3))
        # PSUM: four role-dedicated tags x 2 slots (1 bank each). Each tag's
        # tiles are read by exactly one engine, keeping matmul wait counts <=2.
        psum = ctx.enter_context(tc.tile_pool(name="psum", bufs=2, space="PSUM"))

        # psum->sbuf copies get EXPLICIT engines: every psum tag must have a
        # single reader engine so a consumer matmul's waits stay within the
        # HW limit of 2 sync-wait commands (producer sem + WAR sem).
        def copy_act(dst, src):
            nc.scalar.copy(dst, src)

        def copy_dve(dst, src):
            nc.vector.tensor_copy(dst, src)

        # During the DMA-paced projection phases the attention psum tags are
        # idle; rotating projection psums across all four tags gives 8 chains
        # in flight instead of 2 (Bacc splits any extra semaphore waits).
        ps_rot = ["ps", "s", "pt", "pA"]
        ps_idx = [0]

        def next_ps(cols, nm):
            tag = ps_rot[ps_idx[0] % 4]
            ps_idx[0] += 1
            return psum.tile([128, cols], F32, tag=tag, name=nm, bufs=2)

        # ---- load x^T (8 tiles [128, 640]) interleaved with wv so the V
        # accumulation chains can start as soon as the first pair lands ----
        # PE pstate ramp: the tensor engine reaches full clock 3us after its
        # first instruction. Fire a trivial matmul on locally-memset data
        # immediately (no DMA dependency) so the ramp clock starts at ~t=0.3us
        # instead of ~2.4us when the first loads land (~1us saved).
        zt = consts.tile([1, 8], BF16, tag="zt")
        nc.gpsimd.memset(zt, 0.0)
        warm_ps = psum.tile([1, 8], F32, tag="pt", name="warm", bufs=2)
        nc.tensor.matmul(warm_ps, lhsT=zt[0:1, 0:1], rhs=zt[0:1, 0:8],
                         start=True, stop=True)

        xt = [xpool.tile([128, NTOT], BF16, tag=f"xt{g}", name=f"xt{g}")
              for g in range(DT)]
        wv = [wpool.tile([128, D_MODEL], BF16, tag="w", name=f"wv{g}")
              for g in range(DT)]
        for g in range(DT):
            nc.sync.dma_start(out=xt[g], in_=xT[g * 128:(g + 1) * 128, :])
            nc.sync.dma_start(out=wv[g], in_=wqkvT[g * 128:(g + 1) * 128, 2 * D_MODEL:3 * D_MODEL])

        # ---- constants, queued AFTER the x/wv stream (not needed until the
        # attention phase; keeping them off the head of the DMA queue lets PE
        # start ~1us earlier) ----
        mT = consts.tile([128, STRIP_COLS], BF16, tag="mT")
        nc.sync.dma_start(out=mT, in_=maskT[:, :])
        if use_bqkv or use_bout:
            ones = consts.tile([1, 512], BF16, tag="ones")
            nc.vector.memset(ones, 1.0)
        if use_bqkv:
            bqkv_sb = consts.tile([1, 3 * D_MODEL], BF16, tag="bqkv")
            nc.sync.dma_start(out=bqkv_sb, in_=bqkv[:, :])
        if use_bout:
            bout_sb = consts.tile([1, D_MODEL], BF16, tag="bout")
            nc.sync.dma_start(out=bout_sb, in_=bout[:, :])

        # ---- Phase V: V'[n, h*128+(0:64)] = (x @ wv^T)_h, V'[n, h*128+(64:128)] = 1
        # The interleaved ones-blocks make every PV matmul accumulate the
        # softmax denominators into psum rows 64:128 at zero PE cost. ----
        vt = []
        for n in range(NKB):
            t = vp.tile([128, N_HEADS * 128], BF16, tag=f"v{n}", name=f"v{n}")
            onesview = t[:, :].rearrange("p (h c) -> p h c", c=128)[:, :, D_HEAD:128]
            nc.vector.memset(onesview, 1.0)
            vt.append(t)
        for n in range(NKB):
            for oh in range(2):
                ps = next_ps(512, "psv")
                for g in range(DT):
                    nc.tensor.matmul(
                        ps, lhsT=xt[g][:, n * 128:(n + 1) * 128],
                        rhs=wv[g][:, oh * 512:(oh + 1) * 512],
                        start=(g == 0), stop=(g == DT - 1 and not use_bqkv))
                if use_bqkv:
                    nc.tensor.matmul(
                        ps, lhsT=ones[0:1, 0:128],
                        rhs=bqkv_sb[0:1, 2 * D_MODEL + oh * 512:2 * D_MODEL + (oh + 1) * 512],
                        start=False, stop=True)
                # strided copy: head j of this half -> V' block (8*oh+j)*128
                dst = vt[n][:, oh * 1024:(oh + 1) * 1024].rearrange(
                    "p (h c) -> p h c", c=128)[:, :, 0:D_HEAD]
                src = ps[:, :].rearrange("p (h c) -> p h c", c=D_HEAD)
                copy_dve(dst, src)

        # ---- Phase Q/K + attention, software-pipelined ----
        # Head pairs are processed in order [1..6 in-loop, then 7, then 0]:
        # the LAST pair processed (0) uses qt/kt tiles ready since o=0, so the
        # tail never waits on fresh projection copies; the out-proj chains
        # contract g=0 last for the same reason.
        wq = []
        for g in range(DT):
            t = wpool.tile([128, D_MODEL], BF16, tag="w", name=f"wq{g}")
            nc.sync.dma_start(out=t, in_=wqkvT[g * 128:(g + 1) * 128, 0:D_MODEL])
            wq.append(t)
        wk = []
        for g in range(DT):
            t = wpool.tile([128, D_MODEL], BF16, tag="w", name=f"wk{g}")
            nc.sync.dma_start(out=t, in_=wqkvT[g * 128:(g + 1) * 128, D_MODEL:2 * D_MODEL])
            wk.append(t)

        wo = []
        for g in range(DT):
            t = wpool.tile([128, D_MODEL], BF16, tag="w", name=f"wo{g}")
            nc.sync.dma_start(out=t, in_=woutT[g * 128:(g + 1) * 128, :])
            wo.append(t)

        qt = [qtp.tile([128, NLOC], BF16, tag=f"qt{o}", name=f"qt{o}") for o in range(DT)]
        kt = [ktp.tile([128, NTOT], BF16, tag=f"kt{o}", name=f"kt{o}") for o in range(DT)]
        # Halo keys 0:64 can never be attended (query q sees keys >= q+65, and
        # q >= 0 means key >= 65): skip projecting them, but zero the columns
        # so the kb0 score strip's exp input stays finite (masked afterwards).
        for o in range(DT):
            nc.vector.memset(kt[o][:, 0:64], 0.0)
        ao = [aop.tile([128, NLOC], BF16, tag=f"ao{g}", name=f"ao{g}") for g in range(DT)]

        def emit_qk(o, split_copies=False):
            # QT o-tile: out [128 o, 512 n]; rhs = own rows = xT cols [128, 640)
            # Exp and Copy share an ACT function-set table (act_info.json:
            # exp_and_others), so alternating them costs no table reloads
            cp = copy_act

            def copy_out(dst, src):
                if split_copies:
                    # halve the copies so head 2o's scores (rows 0:64) can
                    # issue after the first half lands (shortens the tail)
                    cp(dst[0:64], src[0:64])
                    cp(dst[64:128], src[64:128])
                else:
                    cp(dst, src)

            ps = next_ps(512, "psq")
            for g in range(DT):
                nc.tensor.matmul(
                    ps, lhsT=wq[g][:, o * 128:(o + 1) * 128],
                    rhs=xt[g][:, HALO:NTOT],
                    start=(g == 0), stop=(g == DT - 1 and not use_bqkv))
            if use_bqkv:
                nc.tensor.matmul(
                    ps, lhsT=bqkv_sb[0:1, o * 128:(o + 1) * 128],
                    rhs=ones[0:1, 0:512], start=False, stop=True)
            copy_out(qt[o], ps)
            # KT o-tile: rows 64:640 (dead halo cols skipped), two N=288 chains
            for (c0, cw) in ((64, 288), (352, 288)):
                ps = next_ps(cw, "pskt")
                for g in range(DT):
                    nc.tensor.matmul(
                        ps[:, 0:cw], lhsT=wk[g][:, o * 128:(o + 1) * 128],
                        rhs=xt[g][:, c0:c0 + cw],
                        start=(g == 0), stop=(g == DT - 1 and not use_bqkv))
                if use_bqkv:
                    nc.tensor.matmul(
                        ps[:, 0:cw], lhsT=bqkv_sb[0:1, D_MODEL + o * 128:D_MODEL + (o + 1) * 128],
                        rhs=ones[0:1, 0:cw], start=False, stop=True)
                copy_out(kt[o][:, c0:c0 + cw], ps[:, 0:cw])

        head_state = {}

        def emit_head_scores(h):
            g = h // 2
            r0 = (h % 2) * D_HEAD          # row offset of head h inside tile g
            # S^T strips into two psum banks. Per bank: the first matmul
            # carries start=True (marks the whole bank pending), later ones
            # first-touch-overwrite their regions, the last carries stop.
            sA = psum.tile([128, 384], F32, tag="s", name="sA", bufs=2)
            sB = psum.tile([128, 384], F32, tag="pt", name="sB", bufs=2)
            kts = lambda kb: kt[g][r0:r0 + D_HEAD, kb * KB:(kb + 1) * KB]
            qts = lambda kb: qt[g][r0:r0 + D_HEAD, STRIP_Q[kb][0]:STRIP_Q[kb][1]]
            mm = nc.tensor.matmul
            mm(sA[:, 0:64], lhsT=kts(0), rhs=qts(0),
               start=True, stop=False, skip_group_check=True)
            mm(sA[:, 64:256], lhsT=kts(1), rhs=qts(1),
               start=False, stop=False, skip_group_check=True)
            mm(sA[:, 256:384], lhsT=kts(4), rhs=qts(4),
               start=False, stop=True, skip_group_check=True)
            mm(sB[:, 0:192], lhsT=kts(2), rhs=qts(2),
               start=True, stop=False, skip_group_check=True)
            mm(sB[:, 192:384], lhsT=kts(3), rhs=qts(3),
               start=False, stop=True, skip_group_check=True)
            # P^T = exp(SCALE * S^T); invalid entries hold finite junk
            # (|SCALE*s| <~ 12, no bf16 overflow), zeroed by the mask below.
            pt_t = work.tile([128, STRIP_COLS], BF16, tag="p", bufs=6, name=f"pt{h}")
            nc.scalar.activation(pt_t[:, 0:384], sA,
                                 mybir.ActivationFunctionType.Exp,
                                 bias=0.0, scale=float(SCALE))
            nc.scalar.activation(pt_t[:, 384:768], sB,
                                 mybir.ActivationFunctionType.Exp,
                                 bias=0.0, scale=float(SCALE))
            # zero the out-of-band entries (Pool; otherwise idle here).
            # Two halves, each pipelined behind its exp, to shorten the
            # exp->mask->PV round trip.
            nc.gpsimd.tensor_mul(pt_t[:, 0:384], pt_t[:, 0:384], mT[:, 0:384])
            nc.gpsimd.tensor_mul(pt_t[:, 384:768], pt_t[:, 384:768], mT[:, 384:768])
            head_state[h] = pt_t

        def emit_head_pv(h):
            g = h // 2
            r0 = (h % 2) * D_HEAD
            pt_t = head_state.pop(h)
            # out'_h [128, 512 q]: rows 0:64 = out_h^T, rows 64:128 = softmax
            # denominators (from the V' ones-blocks). Region order: each
            # column's first writer is kb1/kb4 (kb1 carries the bank start).
            op = psum.tile([128, NLOC], F32, tag="pA", name="opsum", bufs=2)
            vh = lambda kb: vt[kb][:, h * 128:(h + 1) * 128]
            pk = lambda kb: pt_t[:, STRIP_OFF[kb]:STRIP_OFF[kb] + (STRIP_Q[kb][1] - STRIP_Q[kb][0])]
            oq = lambda kb: op[:, STRIP_Q[kb][0]:STRIP_Q[kb][1]]
            mm = nc.tensor.matmul
            # kb1 marks the bank; kb2/3/4 first-touch their fresh regions in
            # ascending order; kb0 accumulates over kb1's region and stops.
            mm(oq(1), lhsT=vh(1), rhs=pk(1),
               start=True, stop=False, skip_group_check=True)
            mm(oq(2), lhsT=vh(2), rhs=pk(2),
               start=False, stop=False, skip_group_check=True)
            mm(oq(3), lhsT=vh(3), rhs=pk(3),
               start=False, stop=False, skip_group_check=True)
            mm(oq(4), lhsT=vh(4), rhs=pk(4),
               start=False, stop=False, skip_group_check=True)
            mm(oq(0), lhsT=vh(0), rhs=pk(0),
               start=False, stop=True, skip_group_check=True)
            # normalize fused into the psum->sbuf copy: ao = out * (1/denom)
            # (DVE divide is rejected by the BIR verifier - no divide ALU;
            # Pool cannot read PSUM; ACT-copy decoupling adds a second psum
            # reader engine whose WAR semaphores cost more than it saves)
            rbb = work.tile([D_HEAD, NLOC], F32, tag="rbb", bufs=2, name="rbb")
            nc.vector.reciprocal(rbb, op[D_HEAD:128, :])
            nc.vector.tensor_mul(ao[g][r0:r0 + D_HEAD, :], op[0:D_HEAD, :], rbb)

        for o in range(DT):
            emit_qk(o)
            if o >= 3:
                emit_head_pv(2 * (o - 2))
                emit_head_pv(2 * (o - 2) + 1)
            if o >= 2:
                emit_head_scores(2 * (o - 1))
                emit_head_scores(2 * (o - 1) + 1)
        # tail: pair 0 (ancient tiles) and pair 7; PE filler (pv 12/13, dmy)
        # covers the exp->mask round trips of the last-scored pairs.
        emit_head_scores(0)
        emit_head_scores(1)
        emit_head_scores(14)
        emit_head_scores(15)
        emit_head_pv(12)
        emit_head_pv(13)
        # 8 trivial matmuls make PE observe every wo DMA queue semaphore
        # here (satisfied by now - wo was prefetched), so phase C's matmuls
        # don't each need a DMA wait slot (HW limit: 2 sync waits per matmul)
        dmy = psum.tile([1, 1], F32, tag="pt", name="dmy", bufs=2)
        for g in range(DT):
            nc.tensor.matmul(dmy, lhsT=wo[g][0:1, 0:1],
                             rhs=wo[g][0:1, 0:1],
                             start=(g == 0), stop=(g == DT - 1))
        emit_head_pv(0)
        emit_head_pv(1)
        emit_head_pv(14)
        emit_head_pv(15)

        # ---- Phase C: out = attnout @ wout^T (+ b_out); g=0 contracted last
        # so the chains only need ao[0] (heads 0/1, finishing on DVE) at the
        # very end of each chain ----
        gorder = [1, 2, 3, 4, 5, 6, 7, 0]

        def outproj_chain(n, c0, cw):
            ps = next_ps(cw, "psc")
            cp = copy_dve
            for gi, g in enumerate(gorder):
                nc.tensor.matmul(
                    ps, lhsT=ao[g][:, n * 128:(n + 1) * 128],
                    rhs=wo[g][:, c0:c0 + cw],
                    start=(gi == 0), stop=(gi == DT - 1 and not use_bout))
            if use_bout:
                nc.tensor.matmul(
                    ps, lhsT=ones[0:1, 0:128],
                    rhs=bout_sb[0:1, c0:c0 + cw],
                    start=False, stop=True)
            ot = outp.tile([128, cw], F32, tag="out", name=f"ot{cw}")
            cp(ot, ps)
            nc.sync.dma_start(
                out=y[n * 128:(n + 1) * 128, c0:c0 + cw], in_=ot)

        for n in range(NQB):
            for oh in range(2):
                if n == NQB - 1 and oh == 1:
                    # split the final chain so the very last copy+DMA is half
                    # as long (shortens the post-PE drain)
                    outproj_chain(n, 512, 256)
                    outproj_chain(n, 768, 256)
                else:
                    outproj_chain(n, oh * 512, 512)

    return nc


_PROG_CACHE: dict = {}


def _get_program(use_bqkv: bool, use_bout: bool) -> bass.Bass:
    key = (use_bqkv, use_bout)
    if key not in _PROG_CACHE:
        nc = _emit_program(use_bqkv, use_bout)
        if not nc.is_finalized():
            nc.finalize()
        _PROG_CACHE[key] = nc
    return _PROG_CACHE[key]


def _build_maskT(core0: bool) -> np.ndarray:
    """0/1 valid bits for the 5 S^T strips, [128 k-rows, 1024 strip cols]."""
    m = np.zeros((128, STRIP_COLS), np.float32)
    i = np.arange(128)[:, None]
    for kb, off in STRIP_OFF.items():
        q0, q1 = STRIP_Q[kb]
        q = np.arange(q0, q1)[None, :]
        k = kb * KB + i
        v = (k >= q + WINDOW + 1) & (k <= q + HALO)
        if core0 and kb == 0:
            v = v & False                    # halo keys are padding on core 0
        m[:, off:off + (q1 - q0)] = v
    return m


def kernel(x, w_qkv, b_qkv, w_out, b_out):
    global LAST_RESULT
    x = np.asarray(x, dtype=np.float32)
    w_qkv = np.asarray(w_qkv, dtype=np.float32)
    b_qkv = np.asarray(b_qkv, dtype=np.float32)
    w_out = np.asarray(w_out, dtype=np.float32)
    b_out = np.asarray(b_out, dtype=np.float32)

    B = x.shape[0]
    assert x.shape == (1, N_SEQ, D_MODEL), x.shape
    xa = x[0]

    use_bqkv = bool(np.any(b_qkv))
    use_bout = bool(np.any(b_out))
    nc = _get_program(use_bqkv, use_bout)

    wqkvT = np.ascontiguousarray(w_qkv.T).astype(NP_BF16)    # [1024, 3072]
    woutT = np.ascontiguousarray(w_out.T).astype(NP_BF16)    # [1024, 1024]
    mT_std = _build_maskT(core0=False).astype(NP_BF16)
    mT_first = _build_maskT(core0=True).astype(NP_BF16)

    in_maps = []
    for c in range(N_CORES):
        s = c * NLOC
        if c == 0:
            blk = np.concatenate([np.zeros((HALO, D_MODEL), np.float32), xa[0:NLOC]], axis=0)
        else:
            blk = xa[s - HALO:s + NLOC]
        im = {
            "xT": np.ascontiguousarray(blk.T).astype(NP_BF16),  # [1024, 640]
            "wqkvT": wqkvT,
            "woutT": woutT,
            "maskT": mT_first if c == 0 else mT_std,
        }
        if use_bqkv:
            im["bqkv"] = b_qkv.reshape(1, 3 * D_MODEL).astype(NP_BF16)
        if use_bout:
            im["bout"] = b_out.reshape(1, D_MODEL).astype(NP_BF16)
        in_maps.append(im)

    res = run_bass_kernel_spmd(nc, in_maps, list(range(N_CORES)))
    LAST_RESULT = res
    out = np.concatenate([res.results[c]["y"] for c in range(N_CORES)], axis=0)
    return out.reshape(B, N_SEQ, D_MODEL)



# revision 6
# speedup vs baseline: 1.0256x; 1.0256x over previous
"""Local causal (sliding-window) attention kernel for Trainium2, SPMD over 8 NeuronCores.

Problem: x [1,4096,1024] -> QKV proj -> 16-head attention with causal window 64
         -> out proj. All fp32 at the interface.

Sharding: sequence-parallel. Core c owns queries [512c, 512c+512). Attention is
local (window 64), so each core only needs a 128-row key/value halo (the
previous 128-token block) in addition to its own 512 rows. Each core computes
its full output rows; host concatenates. No collectives.

All on-chip compute is bf16 (fp32 PSUM accumulation): host casts x/weights to
bf16, which halves HBM traffic (the projection phase is DMA-paced) and runs
every matmul at 1 cycle/row regardless of free-dim size. Measured end-to-end
rel err vs the fp32 reference is ~6e-3.

Attention is computed TRANSPOSED (S^T = K^T-stationary x Q): exp(S^T) is
already P^T, so no PE transposes and no P^T staging copies are needed. V tiles
carry an interleaved ones-block per head ([V_h | 1]), so each PV matmul also
accumulates the softmax denominators into psum rows 64:128 for free; the
normalization (x 1/denom) is fused into the psum->sbuf copy of the attention
output (DVE reciprocal + tensor_mul).

Per-core layouts (host pre-transposes so every DMA is a clean row-major tile):
  xT    [1024 d, 640 n]   x^T for rows [s-128, s+512) (core 0: first 128 zero)
  wqkvT [1024 d, 3072 o]  w_qkv^T
  woutT [1024 d, 1024 o]  w_out^T
  maskT [128, 768]        0/1 valid bits for the 5 S^T strips of one head
                          (per-core data; core 0 zeroes the kb0 strip)

S^T strips are trimmed to the query range that can see the strip's keys
(STRIP_Q, 768 columns total); psum banks A=[kb0, kb1, kb4], B=[kb2, kb3];
the sbuf P^T tile is [A | B] = [128, 768]. Strip kb holds keys
k = 128kb + row; entry (row, q) is valid iff q+65 <= k <= q+128.
"""

from contextlib import ExitStack

import ml_dtypes
import numpy as np

import concourse.bass as bass
import concourse.mybir as mybir
import concourse.tile as tile
from concourse import bacc
from concourse.bass_utils import run_bass_kernel_spmd

F32 = mybir.dt.float32
BF16 = mybir.dt.bfloat16
NP_BF16 = ml_dtypes.bfloat16


D_MODEL = 1024
N_HEADS = 16
D_HEAD = 64
WINDOW = 64
N_SEQ = 4096
N_CORES = 8
NLOC = N_SEQ // N_CORES          # 512 queries per core
HALO = 128                       # one full key block of halo
NTOT = NLOC + HALO               # 640 local rows (keys/values)
QB = 128                         # query block
NQB = NLOC // QB                 # 4 query blocks per core
KB = 128                         # key block
NKB = NTOT // KB                 # 5 key blocks per core
SCALE = 1.0 / np.sqrt(D_HEAD)

DT = D_MODEL // 128              # 8 contraction tiles

# Diagonal 64-query strips: strip u covers queries [64u, 64u+64) whose full
# key windows [q+65, q+128] all fall inside keys [64(u+1), 64(u+1)+128) --
# exactly 128 keys, so every (head, u) score/PV is ONE matmul with no
# cross-strip accumulation. 8 strips x 64 cols = 512 P^T columns per head
# (vs 768 for 128-key-block strips). Entry (k', q') of strip u is valid iff
# 1 <= k' - q' <= 64 -- the same [128, 64] band mask for every strip.
NSTRIP = 8
STRIP_COLS = 512

# exposed for test.py (profiling info)
LAST_RESULT = None


def _emit_program(use_bqkv: bool, use_bout: bool, reps: int = 1) -> bass.Bass:
    # Bacc (not raw Bass): its finalize pipeline splits semaphore waits
    # (move_matmul_waits_to_ldweights / generate_event_semaphores) to satisfy
    # the HW limit of 1 sync wait per instruction.
    nc = bacc.Bacc()

    xT = nc.declare_dram_parameter("xT", [D_MODEL, NTOT], BF16, isOutput=False)
    wqkvT = nc.declare_dram_parameter("wqkvT", [D_MODEL, 3 * D_MODEL], BF16, isOutput=False)
    woutT = nc.declare_dram_parameter("woutT", [D_MODEL, D_MODEL], BF16, isOutput=False)
    maskT = nc.declare_dram_parameter("maskT", [128, STRIP_COLS], BF16, isOutput=False)
    if use_bqkv:
        bqkv = nc.declare_dram_parameter("bqkv", [1, 3 * D_MODEL], BF16, isOutput=False)
    if use_bout:
        bout = nc.declare_dram_parameter("bout", [1, D_MODEL], BF16, isOutput=False)
    y = nc.declare_dram_parameter("y", [NLOC, D_MODEL], F32, isOutput=True)

    with tile.TileContext(nc) as tc:
      for _rep in range(reps):
       with ExitStack() as ctx:
        consts = ctx.enter_context(tc.tile_pool(name="consts", bufs=1))
        xpool = ctx.enter_context(tc.tile_pool(name="xpool", bufs=1))
        wpool = ctx.enter_context(tc.tile_pool(name="wpool", bufs=18))
        qtp = ctx.enter_context(tc.tile_pool(name="qtp", bufs=1))
        ktp = ctx.enter_context(tc.tile_pool(name="ktp", bufs=1))
        vp = ctx.enter_context(tc.tile_pool(name="vp", bufs=1))
        aop = ctx.enter_context(tc.tile_pool(name="aop", bufs=1))
        work = ctx.enter_context(tc.tile_pool(name="work", bufs=2))
        outp = ctx.enter_context(tc.tile_pool(name="outp", bufs=3))
        # PSUM: four role-dedicated tags x 2 slots (1 bank each). Each tag's
        # tiles are read by exactly one engine, keeping matmul wait counts <=2.
        psum = ctx.enter_context(tc.tile_pool(name="psum", bufs=2, space="PSUM"))

        # psum->sbuf copies get EXPLICIT engines: every psum tag must have a
        # single reader engine so a consumer matmul's waits stay within the
        # HW limit of 2 sync-wait commands (producer sem + WAR sem).
        def copy_act(dst, src):
            nc.scalar.copy(dst, src)

        def copy_dve(dst, src):
            nc.vector.tensor_copy(dst, src)

        # During the DMA-paced projection phases the attention psum tags are
        # idle; rotating projection psums across all four tags gives 8 chains
        # in flight instead of 2 (Bacc splits any extra semaphore waits).
        ps_rot = ["ps", "s", "pt", "pA"]
        ps_idx = [0]

        def next_ps(cols, nm):
            tag = ps_rot[ps_idx[0] % 4]
            ps_idx[0] += 1
            return psum.tile([128, cols], F32, tag=tag, name=nm, bufs=2)

        # ---- load x^T (8 tiles [128, 640]) interleaved with wv so the V
        # accumulation chains can start as soon as the first pair lands ----
        # PE pstate ramp: the tensor engine reaches full clock 3us after its
        # first instruction. Fire a trivial matmul on locally-memset data
        # immediately (no DMA dependency) so the ramp clock starts at ~t=0.3us
        # instead of ~2.4us when the first loads land (~1us saved).
        zt = consts.tile([1, 8], BF16, tag="zt")
        nc.gpsimd.memset(zt, 0.0)
        warm_ps = psum.tile([1, 8], F32, tag="pt", name="warm", bufs=2)
        nc.tensor.matmul(warm_ps, lhsT=zt[0:1, 0:1], rhs=zt[0:1, 0:8],
                         start=True, stop=True)

        xt = [xpool.tile([128, NTOT], BF16, tag=f"xt{g}", name=f"xt{g}")
              for g in range(DT)]
        wv = [wpool.tile([128, D_MODEL], BF16, tag="w", name=f"wv{g}")
              for g in range(DT)]
        for g in range(DT):
            nc.sync.dma_start(out=xt[g], in_=xT[g * 128:(g + 1) * 128, :])
            nc.sync.dma_start(out=wv[g], in_=wqkvT[g * 128:(g + 1) * 128, 2 * D_MODEL:3 * D_MODEL])

        # ---- constants, queued AFTER the x/wv stream (not needed until the
        # attention phase; keeping them off the head of the DMA queue lets PE
        # start ~1us earlier) ----
        mT = consts.tile([128, STRIP_COLS], BF16, tag="mT")
        nc.sync.dma_start(out=mT, in_=maskT[:, :])
        if use_bqkv or use_bout:
            ones = consts.tile([1, 512], BF16, tag="ones")
            nc.vector.memset(ones, 1.0)
        if use_bqkv:
            bqkv_sb = consts.tile([1, 3 * D_MODEL], BF16, tag="bqkv")
            nc.sync.dma_start(out=bqkv_sb, in_=bqkv[:, :])
        if use_bout:
            bout_sb = consts.tile([1, D_MODEL], BF16, tag="bout")
            nc.sync.dma_start(out=bout_sb, in_=bout[:, :])

        # ---- Phase V: V'[n, h*128+(0:64)] = (x @ wv^T)_h, V'[n, h*128+(64:128)] = 1
        # The interleaved ones-blocks make every PV matmul accumulate the
        # softmax denominators into psum rows 64:128 at zero PE cost. ----
        vt = []
        for n in range(NKB):
            t = vp.tile([128, N_HEADS * 128], BF16, tag=f"v{n}", name=f"v{n}")
            onesview = t[:, :].rearrange("p (h c) -> p h c", c=128)[:, :, D_HEAD:128]
            nc.vector.memset(onesview, 1.0)
            vt.append(t)
        for n in range(NKB):
            for oh in range(2):
                ps = next_ps(512, "psv")
                for g in range(DT):
                    nc.tensor.matmul(
                        ps, lhsT=xt[g][:, n * 128:(n + 1) * 128],
                        rhs=wv[g][:, oh * 512:(oh + 1) * 512],
                        start=(g == 0), stop=(g == DT - 1 and not use_bqkv))
                if use_bqkv:
                    nc.tensor.matmul(
                        ps, lhsT=ones[0:1, 0:128],
                        rhs=bqkv_sb[0:1, 2 * D_MODEL + oh * 512:2 * D_MODEL + (oh + 1) * 512],
                        start=False, stop=True)
                # strided copy: head j of this half -> V' block (8*oh+j)*128
                dst = vt[n][:, oh * 1024:(oh + 1) * 1024].rearrange(
                    "p (h c) -> p h c", c=128)[:, :, 0:D_HEAD]
                src = ps[:, :].rearrange("p (h c) -> p h c", c=D_HEAD)
                copy_dve(dst, src)
        # Even-u PV strips need V' rows at 64-skewed offsets (64+128j : 192+128j),
        # which straddle two A-tiles; build skewed B-tiles by plain partition-
        # shifted SBUF copies (ones blocks come along for free).
        vtB = []
        for j in range(NKB - 1):
            t = vp.tile([128, N_HEADS * 128], BF16, tag=f"vB{j}", name=f"vB{j}")
            nc.vector.tensor_copy(t[0:64, :], vt[j][64:128, :])
            nc.vector.tensor_copy(t[64:128, :], vt[j + 1][0:64, :])
            vtB.append(t)

        # ---- Phase Q/K + attention, software-pipelined ----
        # Head pairs are processed in order [1..6 in-loop, then 7, then 0]:
        # the LAST pair processed (0) uses qt/kt tiles ready since o=0, so the
        # tail never waits on fresh projection copies; the out-proj chains
        # contract g=0 last for the same reason.
        wq = []
        for g in range(DT):
            t = wpool.tile([128, D_MODEL], BF16, tag="w", name=f"wq{g}")
            nc.sync.dma_start(out=t, in_=wqkvT[g * 128:(g + 1) * 128, 0:D_MODEL])
            wq.append(t)
        wk = []
        for g in range(DT):
            t = wpool.tile([128, D_MODEL], BF16, tag="w", name=f"wk{g}")
            nc.sync.dma_start(out=t, in_=wqkvT[g * 128:(g + 1) * 128, D_MODEL:2 * D_MODEL])
            wk.append(t)

        wo = []
        for g in range(DT):
            t = wpool.tile([128, D_MODEL], BF16, tag="w", name=f"wo{g}")
            nc.sync.dma_start(out=t, in_=woutT[g * 128:(g + 1) * 128, :])
            wo.append(t)

        qt = [qtp.tile([128, NLOC], BF16, tag=f"qt{o}", name=f"qt{o}") for o in range(DT)]
        kt = [ktp.tile([128, NTOT], BF16, tag=f"kt{o}", name=f"kt{o}") for o in range(DT)]
        # Keys 0:64 can never be attended (query q sees keys >= q+65) and no
        # diagonal strip reads them (strip u starts at key 64(u+1) >= 64), so
        # kt cols 0:64 are simply never written.
        ao = [aop.tile([128, NLOC], BF16, tag=f"ao{g}", name=f"ao{g}") for g in range(DT)]

        def emit_qk(o, split_copies=False):
            # QT o-tile: out [128 o, 512 n]; rhs = own rows = xT cols [128, 640)
            # Exp and Copy share an ACT function-set table (act_info.json:
            # exp_and_others), so alternating them costs no table reloads
            cp = copy_act

            def copy_out(dst, src):
                if split_copies:
                    # halve the copies so head 2o's scores (rows 0:64) can
                    # issue after the first half lands (shortens the tail)
                    cp(dst[0:64], src[0:64])
                    cp(dst[64:128], src[64:128])
                else:
                    cp(dst, src)

            ps = next_ps(512, "psq")
            for g in range(DT):
                nc.tensor.matmul(
                    ps, lhsT=wq[g][:, o * 128:(o + 1) * 128],
                    rhs=xt[g][:, HALO:NTOT],
                    start=(g == 0), stop=(g == DT - 1 and not use_bqkv))
            if use_bqkv:
                nc.tensor.matmul(
                    ps, lhsT=bqkv_sb[0:1, o * 128:(o + 1) * 128],
                    rhs=ones[0:1, 0:512], start=False, stop=True)
            copy_out(qt[o], ps)
            # KT o-tile: rows 64:640 (dead halo cols skipped), two N=288 chains
            for (c0, cw) in ((64, 288), (352, 288)):
                ps = next_ps(cw, "pskt")
                for g in range(DT):
                    nc.tensor.matmul(
                        ps[:, 0:cw], lhsT=wk[g][:, o * 128:(o + 1) * 128],
                        rhs=xt[g][:, c0:c0 + cw],
                        start=(g == 0), stop=(g == DT - 1 and not use_bqkv))
                if use_bqkv:
                    nc.tensor.matmul(
                        ps[:, 0:cw], lhsT=bqkv_sb[0:1, D_MODEL + o * 128:D_MODEL + (o + 1) * 128],
                        rhs=ones[0:1, 0:cw], start=False, stop=True)
                copy_out(kt[o][:, c0:c0 + cw], ps[:, 0:cw])

        head_state = {}

        def emit_head_scores(h):
            g = h // 2
            r0 = (h % 2) * D_HEAD          # row offset of head h inside tile g
            # S^T diagonal strips into ONE psum bank [128, 512]. The first
            # matmul carries start=True (marks the whole bank pending), later
            # ones first-touch-overwrite their regions, the last carries stop.
            s_ps = psum.tile([128, STRIP_COLS], F32, tag="s", name="sS", bufs=2)
            mm = nc.tensor.matmul
            for u in range(NSTRIP):
                mm(s_ps[:, 64 * u:64 * u + 64],
                   lhsT=kt[g][r0:r0 + D_HEAD, 64 * (u + 1):64 * (u + 1) + 128],
                   rhs=qt[g][r0:r0 + D_HEAD, 64 * u:64 * u + 64],
                   start=(u == 0), stop=(u == NSTRIP - 1), skip_group_check=True)
            # P^T = exp(SCALE * S^T); invalid entries hold finite junk
            # (|SCALE*s| <~ 12, no bf16 overflow), zeroed by the mask below.
            pt_t = work.tile([128, STRIP_COLS], BF16, tag="p", bufs=6, name=f"pt{h}")
            nc.scalar.activation(pt_t[:, 0:256], s_ps[:, 0:256],
                                 mybir.ActivationFunctionType.Exp,
                                 bias=0.0, scale=float(SCALE))
            nc.scalar.activation(pt_t[:, 256:512], s_ps[:, 256:512],
                                 mybir.ActivationFunctionType.Exp,
                                 bias=0.0, scale=float(SCALE))
            # zero the out-of-band entries (Pool; otherwise idle here).
            # Two halves, each pipelined behind its exp, to shorten the
            # exp->mask->PV round trip.
            nc.gpsimd.tensor_mul(pt_t[:, 0:256], pt_t[:, 0:256], mT[:, 0:256])
            nc.gpsimd.tensor_mul(pt_t[:, 256:512], pt_t[:, 256:512], mT[:, 256:512])
            head_state[h] = pt_t

        def emit_head_pv(h):
            g = h // 2
            r0 = (h % 2) * D_HEAD
            pt_t = head_state.pop(h)
            # out'_h [128, 512 q]: rows 0:64 = out_h^T, rows 64:128 = softmax
            # denominators (from the V' ones-blocks). Each strip u is a single
            # matmul over its 128-key window: odd u hits an aligned A-tile,
            # even u the 64-skewed B-tile.
            op = psum.tile([128, NLOC], F32, tag="pA", name="opsum", bufs=2)
            mm = nc.tensor.matmul
            for u in range(NSTRIP):
                vtile = vt[(u + 1) // 2] if u % 2 == 1 else vtB[u // 2]
                mm(op[:, 64 * u:64 * u + 64],
                   lhsT=vtile[:, h * 128:(h + 1) * 128],
                   rhs=pt_t[:, 64 * u:64 * u + 64],
                   start=(u == 0), stop=(u == NSTRIP - 1), skip_group_check=True)
            # normalize fused into the psum->sbuf copy: ao = out * (1/denom)
            # (DVE divide is rejected by the BIR verifier - no divide ALU;
            # Pool cannot read PSUM; ACT-copy decoupling adds a second psum
            # reader engine whose WAR semaphores cost more than it saves)
            rbb = work.tile([D_HEAD, NLOC], F32, tag="rbb", bufs=2, name="rbb")
            nc.vector.reciprocal(rbb, op[D_HEAD:128, :])
            nc.vector.tensor_mul(ao[g][r0:r0 + D_HEAD, :], op[0:D_HEAD, :], rbb)

        for o in range(DT):
            emit_qk(o)
            if o >= 3:
                emit_head_pv(2 * (o - 2))
                emit_head_pv(2 * (o - 2) + 1)
            if o >= 2:
                emit_head_scores(2 * (o - 1))
                emit_head_scores(2 * (o - 1) + 1)
        # tail: pair 0 (ancient tiles) and pair 7; PE filler (pv 12/13, dmy)
        # covers the exp->mask round trips of the last-scored pairs.
        emit_head_scores(0)
        emit_head_scores(1)
        emit_head_scores(14)
        emit_head_scores(15)
        emit_head_pv(12)
        emit_head_pv(13)
        # 8 trivial matmuls make PE observe every wo DMA queue semaphore
        # here (satisfied by now - wo was prefetched), so phase C's matmuls
        # don't each need a DMA wait slot (HW limit: 2 sync waits per matmul)
        dmy = psum.tile([1, 1], F32, tag="pt", name="dmy", bufs=2)
        for g in range(DT):
            nc.tensor.matmul(dmy, lhsT=wo[g][0:1, 0:1],
                             rhs=wo[g][0:1, 0:1],
                             start=(g == 0), stop=(g == DT - 1))
        emit_head_pv(0)
        emit_head_pv(1)
        emit_head_pv(14)
        emit_head_pv(15)

        # ---- Phase C: out = attnout @ wout^T (+ b_out); g=0 contracted last
        # so the chains only need ao[0] (heads 0/1, finishing on DVE) at the
        # very end of each chain ----
        gorder = [1, 2, 3, 4, 5, 6, 7, 0]

        def outproj_chain(n, c0, cw):
            ps = next_ps(cw, "psc")
            cp = copy_dve
            for gi, g in enumerate(gorder):
                nc.tensor.matmul(
                    ps, lhsT=ao[g][:, n * 128:(n + 1) * 128],
                    rhs=wo[g][:, c0:c0 + cw],
                    start=(gi == 0), stop=(gi == DT - 1 and not use_bout))
            if use_bout:
                nc.tensor.matmul(
                    ps, lhsT=ones[0:1, 0:128],
                    rhs=bout_sb[0:1, c0:c0 + cw],
                    start=False, stop=True)
            ot = outp.tile([128, cw], F32, tag="out", name=f"ot{cw}")
            cp(ot, ps)
            nc.sync.dma_start(
                out=y[n * 128:(n + 1) * 128, c0:c0 + cw], in_=ot)

        for n in range(NQB):
            for oh in range(2):
                if n == NQB - 1 and oh == 1:
                    # split the final chain so the very last copy+DMA is half
                    # as long (shortens the post-PE drain)
                    outproj_chain(n, 512, 256)
                    outproj_chain(n, 768, 256)
                else:
                    outproj_chain(n, oh * 512, 512)

    return nc


_PROG_CACHE: dict = {}


def _get_program(use_bqkv: bool, use_bout: bool) -> bass.Bass:
    key = (use_bqkv, use_bout)
    if key not in _PROG_CACHE:
        nc = _emit_program(use_bqkv, use_bout)
        if not nc.is_finalized():
            nc.finalize()
        _PROG_CACHE[key] = nc
    return _PROG_CACHE[key]


def _build_maskT(core0: bool) -> np.ndarray:
    """0/1 valid bits for the 8 diagonal S^T strips, [128 k-rows, 512 cols].

    Strip u holds keys 64(u+1)+k' vs queries 64u+q'; valid iff
    1 <= k'-q' <= 64. On core 0 the u=0 strip's keys 64:128 (k' < 64) are
    x-padding, so they are masked off too."""
    m = np.zeros((128, STRIP_COLS), np.float32)
    kp = np.arange(128)[:, None]
    qp = np.arange(64)[None, :]
    band = (kp - qp >= 1) & (kp - qp <= WINDOW)
    for u in range(NSTRIP):
        v = band if not (core0 and u == 0) else band & (kp >= 64)
        m[:, 64 * u:64 * u + 64] = v
    return m


def kernel(x, w_qkv, b_qkv, w_out, b_out):
    global LAST_RESULT
    x = np.asarray(x, dtype=np.float32)
    w_qkv = np.asarray(w_qkv, dtype=np.float32)
    b_qkv = np.asarray(b_qkv, dtype=np.float32)
    w_out = np.asarray(w_out, dtype=np.float32)
    b_out = np.asarray(b_out, dtype=np.float32)

    B = x.shape[0]
    assert x.shape == (1, N_SEQ, D_MODEL), x.shape
    xa = x[0]

    use_bqkv = bool(np.any(b_qkv))
    use_bout = bool(np.any(b_out))
    nc = _get_program(use_bqkv, use_bout)

    wqkvT = np.ascontiguousarray(w_qkv.T).astype(NP_BF16)    # [1024, 3072]
    woutT = np.ascontiguousarray(w_out.T).astype(NP_BF16)    # [1024, 1024]
    mT_std = _build_maskT(core0=False).astype(NP_BF16)
    mT_first = _build_maskT(core0=True).astype(NP_BF16)

    in_maps = []
    for c in range(N_CORES):
        s = c * NLOC
        if c == 0:
            blk = np.concatenate([np.zeros((HALO, D_MODEL), np.float32), xa[0:NLOC]], axis=0)
        else:
            blk = xa[s - HALO:s + NLOC]
        im = {
            "xT": np.ascontiguousarray(blk.T).astype(NP_BF16),  # [1024, 640]
            "wqkvT": wqkvT,
            "woutT": woutT,
            "maskT": mT_first if c == 0 else mT_std,
        }
        if use_bqkv:
            im["bqkv"] = b_qkv.reshape(1, 3 * D_MODEL).astype(NP_BF16)
        if use_bout:
            im["bout"] = b_out.reshape(1, D_MODEL).astype(NP_BF16)
        in_maps.append(im)

    res = run_bass_kernel_spmd(nc, in_maps, list(range(N_CORES)))
    LAST_RESULT = res
    out = np.concatenate([res.results[c]["y"] for c in range(N_CORES)], axis=0)
    return out.reshape(B, N_SEQ, D_MODEL)



# revision 11
# speedup vs baseline: 1.0361x; 1.0102x over previous
"""Local causal (sliding-window) attention kernel for Trainium2, SPMD over 8 NeuronCores.

Problem: x [1,4096,1024] -> QKV proj -> 16-head attention with causal window 64
         -> out proj. All fp32 at the interface.

Sharding: sequence-parallel. Core c owns queries [512c, 512c+512). Attention is
local (window 64), so each core only needs a 128-row key/value halo (the
previous 128-token block) in addition to its own 512 rows. Each core computes
its full output rows; host concatenates. No collectives.

All on-chip compute is bf16 (fp32 PSUM accumulation): host casts x/weights to
bf16, which halves HBM traffic (the projection phase is DMA-paced) and runs
every matmul at 1 cycle/row regardless of free-dim size. Measured end-to-end
rel err vs the fp32 reference is ~6e-3.

Attention is computed TRANSPOSED (S^T = K^T-stationary x Q): exp(S^T) is
already P^T, so no PE transposes and no P^T staging copies are needed. V tiles
carry an interleaved ones-block per head ([V_h | 1]), so each PV matmul also
accumulates the softmax denominators into psum rows 64:128 for free; the
normalization (x 1/denom) is fused into the psum->sbuf copy of the attention
output (DVE reciprocal + tensor_mul).

Per-core layouts (host pre-transposes so every DMA is a clean row-major tile):
  xT    [1024 d, 640 n]   x^T for rows [s-128, s+512) (core 0: first 128 zero)
  wqkvT [1024 d, 3072 o]  w_qkv^T
  woutT [1024 d, 1024 o]  w_out^T
  maskT [128, 768]        0/1 valid bits for the 5 S^T strips of one head
                          (per-core data; core 0 zeroes the kb0 strip)

S^T strips are trimmed to the query range that can see the strip's keys
(STRIP_Q, 768 columns total); psum banks A=[kb0, kb1, kb4], B=[kb2, kb3];
the sbuf P^T tile is [A | B] = [128, 768]. Strip kb holds keys
k = 128kb + row; entry (row, q) is valid iff q+65 <= k <= q+128.
"""

from contextlib import ExitStack

import ml_dtypes
import numpy as np

import concourse.bass as bass
import concourse.mybir as mybir
import concourse.tile as tile
from concourse import bacc
from concourse.bass_utils import run_bass_kernel_spmd

F32 = mybir.dt.float32
BF16 = mybir.dt.bfloat16
NP_BF16 = ml_dtypes.bfloat16


D_MODEL = 1024
N_HEADS = 16
D_HEAD = 64
WINDOW = 64
N_SEQ = 4096
N_CORES = 8
NLOC = N_SEQ // N_CORES          # 512 queries per core
HALO = 128                       # one full key block of halo
NTOT = NLOC + HALO               # 640 local rows (keys/values)
QB = 128                         # query block
NQB = NLOC // QB                 # 4 query blocks per core
KB = 128                         # key block
NKB = NTOT // KB                 # 5 key blocks per core
SCALE = 1.0 / np.sqrt(D_HEAD)

DT = D_MODEL // 128              # 8 contraction tiles

# Diagonal 64-query strips: strip u covers queries [64u, 64u+64) whose full
# key windows [q+65, q+128] all fall inside keys [64(u+1), 64(u+1)+128) --
# exactly 128 keys, so every (head, u) score/PV is ONE matmul with no
# cross-strip accumulation. 8 strips x 64 cols = 512 P^T columns per head
# (vs 768 for 128-key-block strips). Entry (k', q') of strip u is valid iff
# 1 <= k' - q' <= 64 -- the same [128, 64] band mask for every strip.
NSTRIP = 8
STRIP_COLS = 512

# exposed for test.py (profiling info)
LAST_RESULT = None


def _emit_program(use_bqkv: bool, use_bout: bool, reps: int = 1) -> bass.Bass:
    # Bacc (not raw Bass): its finalize pipeline splits semaphore waits
    # (move_matmul_waits_to_ldweights / generate_event_semaphores) to satisfy
    # the HW limit of 1 sync wait per instruction.
    nc = bacc.Bacc()

    xT = nc.declare_dram_parameter("xT", [D_MODEL, NTOT], BF16, isOutput=False)
    wqkvT = nc.declare_dram_parameter("wqkvT", [D_MODEL, 3 * D_MODEL], BF16, isOutput=False)
    woutT = nc.declare_dram_parameter("woutT", [D_MODEL, D_MODEL], BF16, isOutput=False)
    maskT = nc.declare_dram_parameter("maskT", [128, STRIP_COLS], BF16, isOutput=False)
    if use_bqkv:
        bqkv = nc.declare_dram_parameter("bqkv", [1, 3 * D_MODEL], BF16, isOutput=False)
    if use_bout:
        bout = nc.declare_dram_parameter("bout", [1, D_MODEL], BF16, isOutput=False)
    y = nc.declare_dram_parameter("y", [NLOC, D_MODEL], BF16, isOutput=True)

    with tile.TileContext(nc) as tc:
      for _rep in range(reps):
       with ExitStack() as ctx:
        consts = ctx.enter_context(tc.tile_pool(name="consts", bufs=1))
        xpool = ctx.enter_context(tc.tile_pool(name="xpool", bufs=1))
        wpool = ctx.enter_context(tc.tile_pool(name="wpool", bufs=18))
        qtp = ctx.enter_context(tc.tile_pool(name="qtp", bufs=1))
        ktp = ctx.enter_context(tc.tile_pool(name="ktp", bufs=1))
        vp = ctx.enter_context(tc.tile_pool(name="vp", bufs=1))
        aop = ctx.enter_context(tc.tile_pool(name="aop", bufs=1))
        work = ctx.enter_context(tc.tile_pool(name="work", bufs=2))
        outp = ctx.enter_context(tc.tile_pool(name="outp", bufs=3))
        # PSUM: four role-dedicated tags x 2 slots (1 bank each). Each tag's
        # tiles are read by exactly one engine, keeping matmul wait counts <=2.
        psum = ctx.enter_context(tc.tile_pool(name="psum", bufs=2, space="PSUM"))

        # psum->sbuf copies get EXPLICIT engines: every psum tag must have a
        # single reader engine so a consumer matmul's waits stay within the
        # HW limit of 2 sync-wait commands (producer sem + WAR sem).
        def copy_act(dst, src):
            nc.scalar.copy(dst, src)

        def copy_dve(dst, src):
            nc.vector.tensor_copy(dst, src)

        # During the DMA-paced projection phases the attention psum tags are
        # idle; rotating projection psums across all four tags gives 8 chains
        # in flight instead of 2 (Bacc splits any extra semaphore waits).
        ps_rot = ["ps", "s", "pt", "pA"]
        ps_idx = [0]

        def next_ps(cols, nm):
            tag = ps_rot[ps_idx[0] % 4]
            ps_idx[0] += 1
            return psum.tile([128, cols], F32, tag=tag, name=nm, bufs=2)

        # ---- load x^T (8 tiles [128, 640]) interleaved with wv so the V
        # accumulation chains can start as soon as the first pair lands ----
        # PE pstate ramp: the tensor engine reaches full clock 3us after its
        # first instruction. Fire a trivial matmul on locally-memset data
        # immediately (no DMA dependency) so the ramp clock starts at ~t=0.3us
        # instead of ~2.4us when the first loads land (~1us saved).
        zt = consts.tile([1, 8], BF16, tag="zt")
        nc.gpsimd.memset(zt, 0.0)
        warm_ps = psum.tile([1, 8], F32, tag="pt", name="warm", bufs=2)
        nc.tensor.matmul(warm_ps, lhsT=zt[0:1, 0:1], rhs=zt[0:1, 0:8],
                         start=True, stop=True)

        xt = [xpool.tile([128, NTOT], BF16, tag=f"xt{g}", name=f"xt{g}")
              for g in range(DT)]
        wv = [wpool.tile([128, D_MODEL], BF16, tag="w", name=f"wv{g}")
              for g in range(DT)]
        for g in range(DT):
            if g == 0:
                # split the head of the stream into small pieces so the first
                # V matmul's inputs (xt0 block 0 + wv0 first half) land early
                nc.sync.dma_start(out=xt[0][:, 0:128], in_=xT[0:128, 0:128])
                nc.sync.dma_start(out=wv[0][:, 0:512],
                                  in_=wqkvT[0:128, 2 * D_MODEL:2 * D_MODEL + 512])
                nc.sync.dma_start(out=xt[0][:, 128:NTOT], in_=xT[0:128, 128:NTOT])
                nc.sync.dma_start(out=wv[0][:, 512:1024],
                                  in_=wqkvT[0:128, 2 * D_MODEL + 512:3 * D_MODEL])
            else:
                nc.sync.dma_start(out=xt[g], in_=xT[g * 128:(g + 1) * 128, :])
                nc.sync.dma_start(out=wv[g], in_=wqkvT[g * 128:(g + 1) * 128, 2 * D_MODEL:3 * D_MODEL])

        # ---- constants, queued AFTER the x/wv stream (not needed until the
        # attention phase; keeping them off the head of the DMA queue lets PE
        # start ~1us earlier) ----
        mT = consts.tile([128, STRIP_COLS], BF16, tag="mT")
        nc.sync.dma_start(out=mT, in_=maskT[:, :])
        if use_bqkv or use_bout:
            ones = consts.tile([1, 512], BF16, tag="ones")
            nc.vector.memset(ones, 1.0)
        if use_bqkv:
            bqkv_sb = consts.tile([1, 3 * D_MODEL], BF16, tag="bqkv")
            nc.sync.dma_start(out=bqkv_sb, in_=bqkv[:, :])
        if use_bout:
            bout_sb = consts.tile([1, D_MODEL], BF16, tag="bout")
            nc.sync.dma_start(out=bout_sb, in_=bout[:, :])

        # ---- Phase V: V'[n, h*128+(0:64)] = (x @ wv^T)_h, V'[n, h*128+(64:128)] = 1
        # The interleaved ones-blocks make every PV matmul accumulate the
        # softmax denominators into psum rows 64:128 at zero PE cost. ----
        vt = []
        for n in range(NKB):
            t = vp.tile([128, N_HEADS * 128], BF16, tag=f"v{n}", name=f"v{n}")
            onesview = t[:, :].rearrange("p (h c) -> p h c", c=128)[:, :, D_HEAD:128]
            nc.vector.memset(onesview, 1.0)
            vt.append(t)
        for n in range(NKB):
            for oh in range(2):
                ps = next_ps(512, "psv")
                for g in range(DT):
                    nc.tensor.matmul(
                        ps, lhsT=xt[g][:, n * 128:(n + 1) * 128],
                        rhs=wv[g][:, oh * 512:(oh + 1) * 512],
                        start=(g == 0), stop=(g == DT - 1 and not use_bqkv))
                if use_bqkv:
                    nc.tensor.matmul(
                        ps, lhsT=ones[0:1, 0:128],
                        rhs=bqkv_sb[0:1, 2 * D_MODEL + oh * 512:2 * D_MODEL + (oh + 1) * 512],
                        start=False, stop=True)
                # strided copy: head j of this half -> V' block (8*oh+j)*128
                dst = vt[n][:, oh * 1024:(oh + 1) * 1024].rearrange(
                    "p (h c) -> p h c", c=128)[:, :, 0:D_HEAD]
                src = ps[:, :].rearrange("p (h c) -> p h c", c=D_HEAD)
                copy_dve(dst, src)
        # Even-u PV strips need V' rows at 64-skewed offsets (64+128j : 192+128j),
        # which straddle two A-tiles; build skewed B-tiles by plain partition-
        # shifted SBUF copies (ones blocks come along for free).
        vtB = []
        for j in range(NKB - 1):
            t = vp.tile([128, N_HEADS * 128], BF16, tag=f"vB{j}", name=f"vB{j}")
            # Pool (gpsimd) is ~90% idle and the copies have huge slack
            # (needed only from the first even-u PV); keeping them off DVE
            # protects the normalize/out-proj copy chains.
            nc.gpsimd.tensor_copy(t[0:64, :], vt[j][64:128, :])
            nc.gpsimd.tensor_copy(t[64:128, :], vt[j + 1][0:64, :])
            vtB.append(t)

        # ---- Phase Q/K + attention, software-pipelined ----
        # Head pairs are processed in order [1..6 in-loop, then 7, then 0]:
        # the LAST pair processed (0) uses qt/kt tiles ready since o=0, so the
        # tail never waits on fresh projection copies; the out-proj chains
        # contract g=0 last for the same reason.
        wq = []
        for g in range(DT):
            t = wpool.tile([128, D_MODEL], BF16, tag="w", name=f"wq{g}")
            nc.sync.dma_start(out=t, in_=wqkvT[g * 128:(g + 1) * 128, 0:D_MODEL])
            wq.append(t)
        wk = []
        for g in range(DT):
            t = wpool.tile([128, D_MODEL], BF16, tag="w", name=f"wk{g}")
            nc.sync.dma_start(out=t, in_=wqkvT[g * 128:(g + 1) * 128, D_MODEL:2 * D_MODEL])
            wk.append(t)

        wo = []
        for g in range(DT):
            t = wpool.tile([128, D_MODEL], BF16, tag="w", name=f"wo{g}")
            nc.sync.dma_start(out=t, in_=woutT[g * 128:(g + 1) * 128, :])
            wo.append(t)

        qt = [qtp.tile([128, NLOC], BF16, tag=f"qt{o}", name=f"qt{o}") for o in range(DT)]
        kt = [ktp.tile([128, NTOT], BF16, tag=f"kt{o}", name=f"kt{o}") for o in range(DT)]
        # Keys 0:64 can never be attended (query q sees keys >= q+65) and no
        # diagonal strip reads them (strip u starts at key 64(u+1) >= 64), so
        # kt cols 0:64 are simply never written.
        ao = [aop.tile([128, NLOC], BF16, tag=f"ao{g}", name=f"ao{g}") for g in range(DT)]

        def emit_qk(o, split_copies=False):
            # QT o-tile: out [128 o, 512 n]; rhs = own rows = xT cols [128, 640)
            # Exp and Copy share an ACT function-set table (act_info.json:
            # exp_and_others), so alternating them costs no table reloads
            cp = copy_act

            def copy_out(dst, src):
                if split_copies:
                    # halve the copies so head 2o's scores (rows 0:64) can
                    # issue after the first half lands (shortens the tail)
                    cp(dst[0:64], src[0:64])
                    cp(dst[64:128], src[64:128])
                else:
                    cp(dst, src)

            ps = next_ps(512, "psq")
            for g in range(DT):
                nc.tensor.matmul(
                    ps, lhsT=wq[g][:, o * 128:(o + 1) * 128],
                    rhs=xt[g][:, HALO:NTOT],
                    start=(g == 0), stop=(g == DT - 1 and not use_bqkv))
            if use_bqkv:
                nc.tensor.matmul(
                    ps, lhsT=bqkv_sb[0:1, o * 128:(o + 1) * 128],
                    rhs=ones[0:1, 0:512], start=False, stop=True)
            copy_out(qt[o], ps)
            # KT o-tile: rows 64:640 (dead halo cols skipped), two N=288 chains
            for (c0, cw) in ((64, 288), (352, 288)):
                ps = next_ps(cw, "pskt")
                for g in range(DT):
                    nc.tensor.matmul(
                        ps[:, 0:cw], lhsT=wk[g][:, o * 128:(o + 1) * 128],
                        rhs=xt[g][:, c0:c0 + cw],
                        start=(g == 0), stop=(g == DT - 1 and not use_bqkv))
                if use_bqkv:
                    nc.tensor.matmul(
                        ps[:, 0:cw], lhsT=bqkv_sb[0:1, D_MODEL + o * 128:D_MODEL + (o + 1) * 128],
                        rhs=ones[0:1, 0:cw], start=False, stop=True)
                copy_out(kt[o][:, c0:c0 + cw], ps[:, 0:cw])

        head_state = {}

        def emit_head_scores(h):
            g = h // 2
            r0 = (h % 2) * D_HEAD          # row offset of head h inside tile g
            # S^T diagonal strips into ONE psum bank [128, 512]. The first
            # matmul carries start=True (marks the whole bank pending), later
            # ones first-touch-overwrite their regions, the last carries stop.
            s_ps = psum.tile([128, STRIP_COLS], F32, tag="s", name="sS", bufs=2)
            mm = nc.tensor.matmul
            for u in range(NSTRIP):
                mm(s_ps[:, 64 * u:64 * u + 64],
                   lhsT=kt[g][r0:r0 + D_HEAD, 64 * (u + 1):64 * (u + 1) + 128],
                   rhs=qt[g][r0:r0 + D_HEAD, 64 * u:64 * u + 64],
                   start=(u == 0), stop=(u == NSTRIP - 1), skip_group_check=True)
            # P^T = exp(SCALE * S^T); invalid entries hold finite junk
            # (|SCALE*s| <~ 12, no bf16 overflow), zeroed by the mask below.
            pt_t = work.tile([128, STRIP_COLS], BF16, tag="p", bufs=6, name=f"pt{h}")
            nc.scalar.activation(pt_t[:, 0:256], s_ps[:, 0:256],
                                 mybir.ActivationFunctionType.Exp,
                                 bias=0.0, scale=float(SCALE))
            nc.scalar.activation(pt_t[:, 256:512], s_ps[:, 256:512],
                                 mybir.ActivationFunctionType.Exp,
                                 bias=0.0, scale=float(SCALE))
            # zero the out-of-band entries (Pool; otherwise idle here).
            # Two halves, each pipelined behind its exp, to shorten the
            # exp->mask->PV round trip.
            nc.gpsimd.tensor_mul(pt_t[:, 0:256], pt_t[:, 0:256], mT[:, 0:256])
            nc.gpsimd.tensor_mul(pt_t[:, 256:512], pt_t[:, 256:512], mT[:, 256:512])
            head_state[h] = pt_t

        def emit_head_pv(h):
            g = h // 2
            r0 = (h % 2) * D_HEAD
            pt_t = head_state.pop(h)
            # out'_h [128, 512 q]: rows 0:64 = out_h^T, rows 64:128 = softmax
            # denominators (from the V' ones-blocks). Each strip u is a single
            # matmul over its 128-key window: odd u hits an aligned A-tile,
            # even u the 64-skewed B-tile.
            op = psum.tile([128, NLOC], F32, tag="pA", name="opsum", bufs=2)
            mm = nc.tensor.matmul
            for u in range(NSTRIP):
                vtile = vt[(u + 1) // 2] if u % 2 == 1 else vtB[u // 2]
                mm(op[:, 64 * u:64 * u + 64],
                   lhsT=vtile[:, h * 128:(h + 1) * 128],
                   rhs=pt_t[:, 64 * u:64 * u + 64],
                   start=(u == 0), stop=(u == NSTRIP - 1), skip_group_check=True)
            # normalize fused into the psum->sbuf copy: ao = out * (1/denom)
            # (DVE divide is rejected by the BIR verifier - no divide ALU;
            # Pool cannot read PSUM; ACT-copy decoupling adds a second psum
            # reader engine whose WAR semaphores cost more than it saves)
            rbb = work.tile([D_HEAD, NLOC], F32, tag="rbb", bufs=2, name="rbb")
            nc.vector.reciprocal(rbb, op[D_HEAD:128, :])
            nc.vector.tensor_mul(ao[g][r0:r0 + D_HEAD, :], op[0:D_HEAD, :], rbb)

        for o in range(DT):
            emit_qk(o)
            if o >= 3:
                emit_head_pv(2 * (o - 2))
                emit_head_pv(2 * (o - 2) + 1)
            if o >= 2:
                emit_head_scores(2 * (o - 1))
                emit_head_scores(2 * (o - 1) + 1)
        # tail: pair 0 (ancient tiles) and pair 7; PE filler (pv 12/13, dmy)
        # covers the exp->mask round trips of the last-scored pairs.
        emit_head_scores(0)
        emit_head_scores(1)
        emit_head_scores(14)
        emit_head_scores(15)
        emit_head_pv(12)
        emit_head_pv(13)
        # 8 trivial matmuls make PE observe every wo DMA queue semaphore
        # here (satisfied by now - wo was prefetched), so phase C's matmuls
        # don't each need a DMA wait slot (HW limit: 2 sync waits per matmul)
        dmy = psum.tile([1, 1], F32, tag="pt", name="dmy", bufs=2)
        for g in range(DT):
            nc.tensor.matmul(dmy, lhsT=wo[g][0:1, 0:1],
                             rhs=wo[g][0:1, 0:1],
                             start=(g == 0), stop=(g == DT - 1))
        emit_head_pv(0)
        emit_head_pv(1)
        emit_head_pv(14)
        emit_head_pv(15)

        # ---- Phase C: out = attnout @ wout^T (+ b_out); g=0 contracted last
        # so the chains only need ao[0] (heads 0/1, finishing on DVE) at the
        # very end of each chain ----
        gorder = [1, 2, 3, 4, 5, 6, 7, 0]

        chain_idx = [0]

        def outproj_chain(n, c0, cw):
            ps = next_ps(cw, "psc")
            cp = copy_dve
            for gi, g in enumerate(gorder):
                nc.tensor.matmul(
                    ps, lhsT=ao[g][:, n * 128:(n + 1) * 128],
                    rhs=wo[g][:, c0:c0 + cw],
                    start=(gi == 0), stop=(gi == DT - 1 and not use_bout))
            if use_bout:
                nc.tensor.matmul(
                    ps, lhsT=ones[0:1, 0:128],
                    rhs=bout_sb[0:1, c0:c0 + cw],
                    start=False, stop=True)
            # y is written in bf16 (host casts back to fp32): halves the DMA
            # bytes and doubles the DVE copy rate on the drain path.
            ot = outp.tile([128, cw], BF16, tag="out", name=f"ot{cw}")
            cp(ot, ps)
            # alternate the y DMAs between the SP and Pool queues so the
            # drain's transfers overlap instead of serializing on SP
            eng = nc.sync if chain_idx[0] % 2 == 0 else nc.gpsimd
            chain_idx[0] += 1
            eng.dma_start(out=y[n * 128:(n + 1) * 128, c0:c0 + cw], in_=ot)

        for n in range(NQB):
            for oh in range(2):
                if n == NQB - 1 and oh == 1:
                    # split the final chain so the very last copy+DMA is small
                    # (shortens the post-PE drain)
                    outproj_chain(n, 512, 384)
                    outproj_chain(n, 896, 128)
                else:
                    outproj_chain(n, oh * 512, 512)

    return nc


_PROG_CACHE: dict = {}


def _get_program(use_bqkv: bool, use_bout: bool) -> bass.Bass:
    key = (use_bqkv, use_bout)
    if key not in _PROG_CACHE:
        nc = _emit_program(use_bqkv, use_bout)
        if not nc.is_finalized():
            nc.finalize()
        _PROG_CACHE[key] = nc
    return _PROG_CACHE[key]


def _build_maskT(core0: bool) -> np.ndarray:
    """0/1 valid bits for the 8 diagonal S^T strips, [128 k-rows, 512 cols].

    Strip u holds keys 64(u+1)+k' vs queries 64u+q'; valid iff
    1 <= k'-q' <= 64. On core 0 the u=0 strip's keys 64:128 (k' < 64) are
    x-padding, so they are masked off too."""
    m = np.zeros((128, STRIP_COLS), np.float32)
    kp = np.arange(128)[:, None]
    qp = np.arange(64)[None, :]
    band = (kp - qp >= 1) & (kp - qp <= WINDOW)
    for u in range(NSTRIP):
        v = band if not (core0 and u == 0) else band & (kp >= 64)
        m[:, 64 * u:64 * u + 64] = v
    return m


def kernel(x, w_qkv, b_qkv, w_out, b_out):
    global LAST_RESULT
    x = np.asarray(x, dtype=np.float32)
    w_qkv = np.asarray(w_qkv, dtype=np.float32)
    b_qkv = np.asarray(b_qkv, dtype=np.float32)
    w_out = np.asarray(w_out, dtype=np.float32)
    b_out = np.asarray(b_out, dtype=np.float32)

    B = x.shape[0]
    assert x.shape == (1, N_SEQ, D_MODEL), x.shape
    xa = x[0]

    use_bqkv = bool(np.any(b_qkv))
    use_bout = bool(np.any(b_out))
    nc = _get_program(use_bqkv, use_bout)

    wqkvT = np.ascontiguousarray(w_qkv.T).astype(NP_BF16)    # [1024, 3072]
    woutT = np.ascontiguousarray(w_out.T).astype(NP_BF16)    # [1024, 1024]
    mT_std = _build_maskT(core0=False).astype(NP_BF16)
    mT_first = _build_maskT(core0=True).astype(NP_BF16)

    in_maps = []
    for c in range(N_CORES):
        s = c * NLOC
        if c == 0:
            blk = np.concatenate([np.zeros((HALO, D_MODEL), np.float32), xa[0:NLOC]], axis=0)
        else:
            blk = xa[s - HALO:s + NLOC]
        im = {
            "xT": np.ascontiguousarray(blk.T).astype(NP_BF16),  # [1024, 640]
            "wqkvT": wqkvT,
            "woutT": woutT,
            "maskT": mT_first if c == 0 else mT_std,
        }
        if use_bqkv:
            im["bqkv"] = b_qkv.reshape(1, 3 * D_MODEL).astype(NP_BF16)
        if use_bout:
            im["bout"] = b_out.reshape(1, D_MODEL).astype(NP_BF16)
        in_maps.append(im)

    res = run_bass_kernel_spmd(nc, in_maps, list(range(N_CORES)))
    LAST_RESULT = res
    out = np.concatenate(
        [res.results[c]["y"].astype(np.float32) for c in range(N_CORES)], axis=0)
    return out.reshape(B, N_SEQ, D_MODEL)

